# revision 14
# baseline (speedup 1.0000x reference)
"""Causal self-attention (B=4, T=2048, D=1024, H=16) on 8 trn2 NeuronCores.

Sharding: core c handles batch b=c//2 and head-group hg=c%2 (8 of 16 heads).
W_q/W_k/W_v are column-sharded per head-group (host-side). W_o is ROW-sharded:
each core computes a partial O-projection y_p[T, D] over its own 512 attention
channels (read straight from SBUF, no collective), and the host sums the two
partials of each batch pair. No device collectives at all.

All matmuls run in bf16 with fp32 PSUM accumulation. Softmax is computed
without max-subtraction (scores are O(1) here; exp is safe in fp32/bf16),
with the denominator obtained from an extra ones-column appended to V.

Startup is pipelined: x arrives host-pre-chunked in four contiguous T-chunks
interleaved behind W_q/W_k so the first QK tile starts ~12us in at full HBM
bandwidth; QK matmuls skip the fully-masked columns of diagonal tiles (one
strided exp covers both heads' valid spans there).
"""

import os
import sys

for _p in ("/opt/trn_rl_repo", "/root/.axon_site/_ro/trn_rl_repo"):
    if os.path.isdir(_p) and _p not in sys.path:
        sys.path.insert(0, _p)

import ml_dtypes
import numpy as np

import concourse.bass as bass  # noqa: F401  (AP helpers)
import concourse.mybir as mybir
import concourse.tile as tile
from concourse.bacc import Bacc
from concourse.bass_utils import run_bass_kernel_spmd
from concourse.masks import make_identity

B = 4
D = 1024
H = 16
DH = 64
N_CORES = 8
HG = 2              # tensor-parallel degree within a batch (head groups)
CL = D // HG        # 512 local channels (8 heads) per core
SCALE = 1.0 / 8.0   # 1 / sqrt(DH)

BF16 = mybir.dt.bfloat16
F32 = mybir.dt.float32
NPBF16 = ml_dtypes.bfloat16
EXP = mybir.ActivationFunctionType.Exp

# Default sequence length; build_nc(T) is parametric for testing.
T_FULL = 2048

N_WARMUP = 20


def build_nc(T):
    NT = T // 128          # t-tiles
    ND = D // 128          # d-tiles (8)
    NCT = CL // 128        # local c-tiles / head pairs (4)
    NJ = T // 512          # tq chunks
    assert T % 512 == 0

    nc = Bacc(None)
    xC = nc.dram_tensor("xC", [D * (T // 512), 512], BF16, kind="ExternalInput")
    wqT = nc.dram_tensor("wqT", [D, CL], BF16, kind="ExternalInput")
    wkT = nc.dram_tensor("wkT", [D, CL], BF16, kind="ExternalInput")
    wvT = nc.dram_tensor("wvT", [D, CL], BF16, kind="ExternalInput")
    woR = nc.dram_tensor("woR", [CL, D], BF16, kind="ExternalInput")
    mask = nc.dram_tensor("mask", [128, 128], BF16, kind="ExternalInput")
    y = nc.dram_tensor("y", [T, D], BF16, kind="ExternalOutput")

    with tile.TileContext(nc) as tc:
        with (
            tc.tile_pool(name="const", bufs=1) as constp,
            tc.tile_pool(name="wo", bufs=1) as wop,
            tc.tile_pool(name="qk", bufs=1) as qkp,
            tc.tile_pool(name="vaug", bufs=1) as vaugp,
            tc.tile_pool(name="outT", bufs=1) as outTp,
        ):
            mask_sb = constp.tile([128, 128], BF16, tag="mask", name="maskt")
            nc.sync.dma_start(mask_sb[:], mask[:])
            ident = constp.tile([128, 128], BF16, tag="ident", name="ident")
            make_identity(nc, ident[:])

            qt_sb = [qkp.tile([128, T], BF16, tag=f"q{ct}", name=f"q{ct}") for ct in range(NCT)]
            kt_sb = [qkp.tile([128, T], BF16, tag=f"k{ct}", name=f"k{ct}") for ct in range(NCT)]
            vaug_sb = [vaugp.tile([128, 8 * 65], BF16, tag=f"v{tt}", name=f"v{tt}") for tt in range(NT)]
            outT_sb = [outTp.tile([128, T], BF16, tag=f"o{ct}", name=f"o{ct}") for ct in range(NCT)]

            with (
                tc.tile_pool(name="xtw", bufs=1) as xtwp,
                tc.tile_pool(name="qkvps", bufs=1, space="PSUM") as qkvpsp,
            ):
                # PE warmup: keep the systolic array active through the
                # initial DMA window (raises the p-state before real work).
                junk = xtwp.tile([128, 512], BF16, tag="junk", name="junk")
                nc.vector.memset(junk[:], 0.5)

                # x arrives host-pre-chunked (chunk-major [NJ*D, 512]), so
                # every [128, 512] chunk tile is one fully-contiguous 128KB
                # DMA: peak HBM bandwidth AND per-chunk dependency
                # granularity (the first QK tile only waits on chunk 0).
                xt_sb = [[xtwp.tile([128, 512], BF16, tag=f"x{dt}_{c}", name=f"x{dt}_{c}")
                          for c in range(NJ)] for dt in range(ND)]
                wq_sb, wk_sb, wv_sb = [], [], []
                for wname, w_dram, lst in (("wq", wqT, wq_sb), ("wk", wkT, wk_sb),
                                           ("wv", wvT, wv_sb)):
                    for dt in range(ND):
                        lst.append(xtwp.tile([128, CL], BF16, tag=f"{wname}{dt}",
                                             name=f"{wname}{dt}"))
                wo_sb = [wop.tile([128, D], BF16, tag=f"wo{ct}", name=f"wo{ct}")
                         for ct in range(NCT)]

                # DMA order: (wq, wk, x-chunk0) interleaved per d-tile, then
                # wv, then x chunks 1..3, then woR. Single sync queue keeps
                # the order; consumers wake as their tiles land.
                def xdma(dt, c):
                    nc.sync.dma_start(
                        xt_sb[dt][c][:],
                        xC[c * D + dt * 128:c * D + (dt + 1) * 128, :])

                for dt in range(ND):
                    nc.sync.dma_start(wq_sb[dt][:], wqT[dt * 128:(dt + 1) * 128, :])
                    nc.sync.dma_start(wk_sb[dt][:], wkT[dt * 128:(dt + 1) * 128, :])
                    xdma(dt, 0)
                for dt in range(ND):
                    xdma(dt, 1)
                for dt in range(ND):
                    nc.sync.dma_start(wv_sb[dt][:], wvT[dt * 128:(dt + 1) * 128, :])
                for c in range(2, NJ):
                    for dt in range(ND):
                        xdma(dt, c)
                for ct in range(NCT):
                    nc.sync.dma_start(wo_sb[ct][:], woR[ct * 128:(ct + 1) * 128, :])

                with tc.tile_pool(name="warmps", bufs=1, space="PSUM") as warmpsp:
                    wps = warmpsp.tile([128, 512], F32, tag="wps", name="wps")
                    for _ in range(N_WARMUP):
                        nc.tensor.matmul(wps[:], junk[:, 0:128], junk[:],
                                         start=True, stop=True)

                # ---- QKV emit helpers (upfront + attention fillers) ----
                def emit_qt(w_sb, dst, ct, tq):
                    ps = qkvpsp.tile([128, 512], F32, tag="qkvps", name="qkvps")
                    for dt in range(ND):
                        nc.tensor.matmul(
                            ps[:],
                            w_sb[dt][:, ct * 128:(ct + 1) * 128],
                            xt_sb[dt][tq][:],
                            start=(dt == 0), stop=(dt == ND - 1),
                        )
                    nc.vector.tensor_copy(dst[ct][:, tq * 512:(tq + 1) * 512], ps[:])

                def emit_v(tt):
                    ps = qkvpsp.tile([128, 512], F32, tag="qkvps", name="qkvps")
                    for dt in range(ND):
                        nc.tensor.matmul(
                            ps[:],
                            xt_sb[dt][tt // 4][:, (tt % 4) * 128:(tt % 4 + 1) * 128],
                            wv_sb[dt][:],
                            start=(dt == 0), stop=(dt == ND - 1),
                        )
                    nc.vector.memset(vaug_sb[tt][:], 1.0)
                    dst = vaug_sb[tt][:].rearrange("p (h e) -> p h e", e=65)[:, :, 0:64]
                    src = ps[:].rearrange("p (h e) -> p h e", e=64)
                    nc.vector.tensor_copy(dst, src)

                # upfront: only what iteration (hp=0, J=0) needs
                emit_qt(wq_sb, qt_sb, 0, 0)
                emit_qt(wk_sb, kt_sb, 0, 0)
                for tt in range(4):
                    emit_v(tt)

                # deferred QKV work, tagged with the work-iteration index
                # (hp*NJ+J) that first consumes it
                fillers = []  # (deadline_idx, closure)
                for tt in range(4, NT):
                    fillers.append((tt // 4, lambda tt=tt: emit_v(tt)))
                for ct in range(1, NCT):
                    for c in range(NJ):
                        fillers.append(
                            (ct * NJ + c, lambda ct=ct, c=c: emit_qt(wq_sb, qt_sb, ct, c)))
                        fillers.append(
                            (ct * NJ + c, lambda ct=ct, c=c: emit_qt(wk_sb, kt_sb, ct, c)))
                for c in range(1, NJ):
                    fillers.append((c, lambda c=c: emit_qt(wq_sb, qt_sb, 0, c)))
                    fillers.append((c, lambda c=c: emit_qt(wk_sb, kt_sb, 0, c)))
                fillers.sort(key=lambda x: x[0])

                # ---------------- Attention ----------------
                with (
                    tc.tile_pool(name="att", bufs=34) as attp,
                    tc.tile_pool(name="on", bufs=3) as onp,
                    tc.tile_pool(name="rc", bufs=4) as rcp,
                    tc.tile_pool(name="stps", bufs=2, space="PSUM") as stpsp,
                    tc.tile_pool(name="avps", bufs=2, space="PSUM") as avpsp,
                    tc.tile_pool(name="tpps", bufs=1, space="PSUM") as tppsp,
                ):
                    def emit_qk_tile(hp, J, i, atts):
                        st = stpsp.tile([128, 1024], F32, tag="st", name="st")
                        k = i - 4 * J
                        o = max(0, k) * 128
                        for h in range(2):
                            # skip the fully-masked (tq < tk) columns of
                            # diagonal tiles
                            nc.tensor.matmul(
                                st[:, h * 512 + o:(h + 1) * 512],
                                kt_sb[hp][h * 64:(h + 1) * 64, i * 128:(i + 1) * 128],
                                qt_sb[hp][h * 64:(h + 1) * 64, J * 512 + o:(J + 1) * 512],
                                start=True, stop=True, tile_position=(h * 64, 0),
                            )
                        att = attp.tile([128, 1024], BF16, tag="att", name="att")
                        if o == 0:
                            nc.scalar.activation(att[:], st[:], EXP, scale=SCALE)
                        else:
                            # one strided exp covering both heads' valid spans
                            a3 = att[:].rearrange("p (h e) -> p h e", e=512)[:, :, o:512]
                            s3 = st[:].rearrange("p (h e) -> p h e", e=512)[:, :, o:512]
                            nc.scalar.activation(a3, s3, EXP, scale=SCALE)
                        if k >= 0:  # diagonal 128-block: keep tk <= tq
                            for h in range(2):
                                lo = h * 512 + k * 128
                                nc.gpsimd.tensor_mul(
                                    att[:, lo:lo + 128], att[:, lo:lo + 128], mask_sb[:]
                                )
                        atts.append(att)

                    def emit_av_mms(hp, J, jj, h, av, atts):
                        jq = 4 * J + jj
                        for i in range(jq + 1):
                            lhsT = atts[i][:, h * 512 + jj * 128:h * 512 + (jj + 1) * 128]
                            hl = hp * 2 + h
                            nc.tensor.matmul(
                                av[:, h * 65:(h + 1) * 65],
                                lhsT,
                                vaug_sb[i][:, hl * 65:(hl + 1) * 65],
                                start=(i == 0), stop=(i == jq),
                            )

                    def emit_av_finish(hp, J, jj, av):
                        onorm = onp.tile([128, 128], BF16, tag="on", name="on")
                        rc = rcp.tile([128, 2], F32, tag="rc", name="rc")
                        # both heads' denominators in one strided reciprocal
                        nc.vector.reciprocal(
                            rc[:], av[:].rearrange("p (h e) -> p h e", e=65)[:, :, 64:65])
                        for h in range(2):
                            nc.vector.tensor_scalar_mul(
                                onorm[:, h * 64:(h + 1) * 64],
                                av[:, h * 65:h * 65 + 64],
                                rc[:, h:h + 1],
                            )
                        tp = tppsp.tile([128, 128], BF16, tag="tp", name="tp")
                        nc.tensor.transpose(tp[:], onorm[:], ident[:])
                        nc.vector.tensor_copy(
                            outT_sb[hp][:, J * 512 + jj * 128:J * 512 + (jj + 1) * 128],
                            tp[:],
                        )

                    def make_av_items(hp, J, atts):
                        items = []
                        for jj in range(4):
                            av = avpsp.tile([128, 130], F32, tag="av", name="av")
                            for h in range(2):
                                items.append(
                                    lambda hp=hp, J=J, jj=jj, h=h, av=av, atts=atts:
                                    emit_av_mms(hp, J, jj, h, av, atts)
                                )
                            items.append(
                                lambda hp=hp, J=J, jj=jj, av=av:
                                emit_av_finish(hp, J, jj, av)
                            )
                        return items

                    work = [(hp, J) for hp in range(NCT) for J in range(NJ)]
                    av_queue = []
                    fpos = 0
                    for idx, (hp, J) in enumerate(work):
                        n_tk = 4 * J + 4
                        atts = []
                        # interleave: previous iteration's AV work + QKV
                        # fillers due up to two iterations out
                        due = []
                        while fpos < len(fillers) and fillers[fpos][0] <= idx + 2:
                            due.append(fillers[fpos][1])
                            fpos += 1
                        mixed = []
                        na, nd = len(av_queue), len(due)
                        ai = di = 0
                        for s in range(na + nd):
                            if ai * nd <= di * na and ai < na:
                                mixed.append(av_queue[ai]); ai += 1
                            elif di < nd:
                                mixed.append(due[di]); di += 1
                            else:
                                mixed.append(av_queue[ai]); ai += 1
                        total = len(mixed)
                        done = 0
                        for i in range(n_tk):
                            emit_qk_tile(hp, J, i, atts)
                            want = ((i + 1) * total) // n_tk
                            while done < want:
                                mixed[done]()
                                done += 1
                        while done < total:
                            mixed[done]()
                            done += 1
                        av_queue = make_av_items(hp, J, atts)
                    for c in av_queue:
                        c()

            # ---------------- Output projection (partial, own channels) ----
            with (
                tc.tile_pool(name="ysb", bufs=3) as ysbp,
                tc.tile_pool(name="yps", bufs=4, space="PSUM") as ypsp,
            ):
                for tt in range(NT):
                    yps = ypsp.tile([128, 1024], F32, tag="yps", name="yps")
                    for half in range(2):
                        for ct in range(NCT):
                            nc.tensor.matmul(
                                yps[:, half * 512:(half + 1) * 512],
                                outT_sb[ct][:, tt * 128:(tt + 1) * 128],
                                wo_sb[ct][:, half * 512:(half + 1) * 512],
                                start=(ct == 0), stop=(ct == NCT - 1),
                            )
                    ysb = ysbp.tile([128, 1024], BF16, tag="ysb", name="ysb")
                    nc.vector.tensor_copy(ysb[:], yps[:])
                    nc.sync.dma_start(y[tt * 128:(tt + 1) * 128, :], ysb[:])

    nc.compile()
    return nc


_NC_CACHE = {}


def _get_nc(T):
    if T not in _NC_CACHE:
        _NC_CACHE[T] = build_nc(T)
    return _NC_CACHE[T]


def shard_inputs(x, W_q, W_k, W_v, W_o):
    """Host-side sharding: per-core input dicts (bf16, transposed)."""
    T = x.shape[1]
    tri = np.triu(np.ones((128, 128), np.float32)).astype(NPBF16)
    in_maps = []
    for c in range(N_CORES):
        b, hg = c // 2, c % 2
        cs = slice(hg * CL, (hg + 1) * CL)
        in_maps.append({
            "xC": np.ascontiguousarray(
                x[b].T.reshape(D, T // 512, 512).transpose(1, 0, 2)
                .reshape(-1, 512)).astype(NPBF16),
            "wqT": np.ascontiguousarray(W_q[cs, :].T).astype(NPBF16),
            "wkT": np.ascontiguousarray(W_k[cs, :].T).astype(NPBF16),
            "wvT": np.ascontiguousarray(W_v[cs, :].T).astype(NPBF16),
            "woR": np.ascontiguousarray(W_o[:, cs].T).astype(NPBF16),
            "mask": tri,
        })
    return in_maps


def assemble_output(results, T):
    y = np.zeros((B, T, D), np.float32)
    for c in range(0, N_CORES, 2):
        b = c // 2
        y[b] = (results[c]["y"].astype(np.float32)
                + results[c + 1]["y"].astype(np.float32))
    return y


def kernel(x, W_q, W_k, W_v, W_o, _trace=False):
    x = np.asarray(x, dtype=np.float32)
    W_q = np.asarray(W_q, dtype=np.float32)
    W_k = np.asarray(W_k, dtype=np.float32)
    W_v = np.asarray(W_v, dtype=np.float32)
    W_o = np.asarray(W_o, dtype=np.float32)
    T = x.shape[1]
    nc = _get_nc(T)
    in_maps = shard_inputs(x, W_q, W_k, W_v, W_o)
    res = run_bass_kernel_spmd(
        nc, in_maps, core_ids=list(range(N_CORES)), trace=_trace
    )
    out = assemble_output(res.results, T)
    if _trace:
        return out, res
    return out


# revision 16
# speedup vs baseline: 1.0020x; 1.0020x over previous
"""Causal self-attention (B=4, T=2048, D=1024, H=16) on 8 trn2 NeuronCores.

Sharding: core c handles batch b=c//2 and head-group hg=c%2 (8 of 16 heads).
W_q/W_k/W_v are column-sharded per head-group (host-side). W_o is ROW-sharded:
each core computes a partial O-projection y_p[T, D] over its own 512 attention
channels (read straight from SBUF, no collective), and the host sums the two
partials of each batch pair. No device collectives at all.

All matmuls run in bf16 with fp32 PSUM accumulation. Softmax is computed
without max-subtraction (scores are O(1) here; exp is safe in fp32/bf16),
with the denominator obtained from an extra ones-column appended to V.

Startup is pipelined: x arrives host-pre-chunked in four contiguous T-chunks
interleaved behind W_q/W_k so the first QK tile starts ~12us in at full HBM
bandwidth; QK matmuls skip the fully-masked columns of diagonal tiles (one
strided exp covers both heads' valid spans there).
"""

import os
import sys

for _p in ("/opt/trn_rl_repo", "/root/.axon_site/_ro/trn_rl_repo"):
    if os.path.isdir(_p) and _p not in sys.path:
        sys.path.insert(0, _p)

import ml_dtypes
import numpy as np

import concourse.bass as bass  # noqa: F401  (AP helpers)
import concourse.mybir as mybir
import concourse.tile as tile
from concourse.bacc import Bacc
from concourse.bass_utils import run_bass_kernel_spmd
from concourse.masks import make_identity

B = 4
D = 1024
H = 16
DH = 64
N_CORES = 8
HG = 2              # tensor-parallel degree within a batch (head groups)
CL = D // HG        # 512 local channels (8 heads) per core
SCALE = 1.0 / 8.0   # 1 / sqrt(DH)

BF16 = mybir.dt.bfloat16
F32 = mybir.dt.float32
NPBF16 = ml_dtypes.bfloat16
EXP = mybir.ActivationFunctionType.Exp

# Default sequence length; build_nc(T) is parametric for testing.
T_FULL = 2048

N_WARMUP = 20


def build_nc(T):
    NT = T // 128          # t-tiles
    ND = D // 128          # d-tiles (8)
    NCT = CL // 128        # local c-tiles / head pairs (4)
    NJ = T // 512          # tq chunks
    assert T % 512 == 0

    nc = Bacc(None)
    xC = nc.dram_tensor("xC", [D * (T // 512), 512], BF16, kind="ExternalInput")
    wqT = nc.dram_tensor("wqT", [D, CL], BF16, kind="ExternalInput")
    wkT = nc.dram_tensor("wkT", [D, CL], BF16, kind="ExternalInput")
    wvT = nc.dram_tensor("wvT", [D, CL], BF16, kind="ExternalInput")
    woR = nc.dram_tensor("woR", [CL, D], BF16, kind="ExternalInput")
    mask = nc.dram_tensor("mask", [128, 128], BF16, kind="ExternalInput")
    y = nc.dram_tensor("y", [T, D], BF16, kind="ExternalOutput")

    with tile.TileContext(nc) as tc:
        with (
            tc.tile_pool(name="const", bufs=1) as constp,
            tc.tile_pool(name="wo", bufs=1) as wop,
            tc.tile_pool(name="qk", bufs=1) as qkp,
            tc.tile_pool(name="vaug", bufs=1) as vaugp,
            tc.tile_pool(name="outT", bufs=1) as outTp,
        ):
            mask_sb = constp.tile([128, 128], BF16, tag="mask", name="maskt")
            nc.sync.dma_start(mask_sb[:], mask[:])
            ident = constp.tile([128, 128], BF16, tag="ident", name="ident")
            make_identity(nc, ident[:])

            qt_sb = [qkp.tile([128, T], BF16, tag=f"q{ct}", name=f"q{ct}") for ct in range(NCT)]
            kt_sb = [qkp.tile([128, T], BF16, tag=f"k{ct}", name=f"k{ct}") for ct in range(NCT)]
            vaug_sb = [vaugp.tile([128, 8 * 65], BF16, tag=f"v{tt}", name=f"v{tt}") for tt in range(NT)]
            outT_sb = [outTp.tile([128, T], BF16, tag=f"o{ct}", name=f"o{ct}") for ct in range(NCT)]

            with (
                tc.tile_pool(name="xtw", bufs=1) as xtwp,
                tc.tile_pool(name="qkvps", bufs=1, space="PSUM") as qkvpsp,
            ):
                # PE warmup: keep the systolic array active through the
                # initial DMA window (raises the p-state before real work).
                junk = xtwp.tile([128, 512], BF16, tag="junk", name="junk")
                nc.vector.memset(junk[:], 0.5)

                # x arrives host-pre-chunked (chunk-major [NJ*D, 512]), so
                # every [128, 512] chunk tile is one fully-contiguous 128KB
                # DMA: peak HBM bandwidth AND per-chunk dependency
                # granularity (the first QK tile only waits on chunk 0).
                xt_sb = [[xtwp.tile([128, 512], BF16, tag=f"x{dt}_{c}", name=f"x{dt}_{c}")
                          for c in range(NJ)] for dt in range(ND)]
                wq_sb, wk_sb, wv_sb = [], [], []
                for wname, w_dram, lst in (("wq", wqT, wq_sb), ("wk", wkT, wk_sb),
                                           ("wv", wvT, wv_sb)):
                    for dt in range(ND):
                        lst.append(xtwp.tile([128, CL], BF16, tag=f"{wname}{dt}",
                                             name=f"{wname}{dt}"))
                wo_sb = [wop.tile([128, D], BF16, tag=f"wo{ct}", name=f"wo{ct}")
                         for ct in range(NCT)]

                # DMA order: (wq, wk, x-chunk0) interleaved per d-tile, then
                # wv, then x chunks 1..3, then woR. Single sync queue keeps
                # the order; consumers wake as their tiles land.
                # DMA triggers cost ~0.6us of queue occupancy each, so a
                # single queue caps the input load at ~200GB/s. Round-robin
                # the triggers over four engine queues (all near-idle during
                # the load) so the DMA engines, not trigger issue, set the
                # pace.
                _dq = [nc.sync, nc.gpsimd, nc.scalar]
                _dqi = [0]

                def dq_dma(dst, src_ap):
                    _dq[_dqi[0] % len(_dq)].dma_start(dst, src_ap)
                    _dqi[0] += 1

                def xdma(dt, c):
                    dq_dma(xt_sb[dt][c][:],
                           xC[c * D + dt * 128:c * D + (dt + 1) * 128, :])

                for dt in range(ND):
                    dq_dma(wq_sb[dt][:], wqT[dt * 128:(dt + 1) * 128, :])
                    dq_dma(wk_sb[dt][:], wkT[dt * 128:(dt + 1) * 128, :])
                    xdma(dt, 0)
                for dt in range(ND):
                    xdma(dt, 1)
                for dt in range(ND):
                    dq_dma(wv_sb[dt][:], wvT[dt * 128:(dt + 1) * 128, :])
                for c in range(2, NJ):
                    for dt in range(ND):
                        xdma(dt, c)
                for ct in range(NCT):
                    dq_dma(wo_sb[ct][:], woR[ct * 128:(ct + 1) * 128, :])

                with tc.tile_pool(name="warmps", bufs=1, space="PSUM") as warmpsp:
                    wps = warmpsp.tile([128, 512], F32, tag="wps", name="wps")
                    for _ in range(N_WARMUP):
                        nc.tensor.matmul(wps[:], junk[:, 0:128], junk[:],
                                         start=True, stop=True)

                # ---- QKV emit helpers (upfront + attention fillers) ----
                def emit_qt(w_sb, dst, ct, tq):
                    ps = qkvpsp.tile([128, 512], F32, tag="qkvps", name="qkvps")
                    for dt in range(ND):
                        nc.tensor.matmul(
                            ps[:],
                            w_sb[dt][:, ct * 128:(ct + 1) * 128],
                            xt_sb[dt][tq][:],
                            start=(dt == 0), stop=(dt == ND - 1),
                        )
                    nc.vector.tensor_copy(dst[ct][:, tq * 512:(tq + 1) * 512], ps[:])

                def emit_v(tt):
                    ps = qkvpsp.tile([128, 512], F32, tag="qkvps", name="qkvps")
                    for dt in range(ND):
                        nc.tensor.matmul(
                            ps[:],
                            xt_sb[dt][tt // 4][:, (tt % 4) * 128:(tt % 4 + 1) * 128],
                            wv_sb[dt][:],
                            start=(dt == 0), stop=(dt == ND - 1),
                        )
                    nc.vector.memset(vaug_sb[tt][:], 1.0)
                    dst = vaug_sb[tt][:].rearrange("p (h e) -> p h e", e=65)[:, :, 0:64]
                    src = ps[:].rearrange("p (h e) -> p h e", e=64)
                    nc.vector.tensor_copy(dst, src)

                # upfront: only what iteration (hp=0, J=0) needs
                emit_qt(wq_sb, qt_sb, 0, 0)
                emit_qt(wk_sb, kt_sb, 0, 0)
                for tt in range(4):
                    emit_v(tt)

                # deferred QKV work, tagged with the work-iteration index
                # (hp*NJ+J) that first consumes it
                fillers = []  # (deadline_idx, closure)
                for tt in range(4, NT):
                    fillers.append((tt // 4, lambda tt=tt: emit_v(tt)))
                for ct in range(1, NCT):
                    for c in range(NJ):
                        fillers.append(
                            (ct * NJ + c, lambda ct=ct, c=c: emit_qt(wq_sb, qt_sb, ct, c)))
                        fillers.append(
                            (ct * NJ + c, lambda ct=ct, c=c: emit_qt(wk_sb, kt_sb, ct, c)))
                for c in range(1, NJ):
                    fillers.append((c, lambda c=c: emit_qt(wq_sb, qt_sb, 0, c)))
                    fillers.append((c, lambda c=c: emit_qt(wk_sb, kt_sb, 0, c)))
                fillers.sort(key=lambda x: x[0])

                # ---------------- Attention ----------------
                with (
                    tc.tile_pool(name="att", bufs=34) as attp,
                    tc.tile_pool(name="on", bufs=3) as onp,
                    tc.tile_pool(name="rc", bufs=4) as rcp,
                    tc.tile_pool(name="stps", bufs=2, space="PSUM") as stpsp,
                    tc.tile_pool(name="avps", bufs=2, space="PSUM") as avpsp,
                    tc.tile_pool(name="tpps", bufs=1, space="PSUM") as tppsp,
                ):
                    def emit_qk_tile(hp, J, i, atts):
                        st = stpsp.tile([128, 1024], F32, tag="st", name="st")
                        k = i - 4 * J
                        o = max(0, k) * 128
                        for h in range(2):
                            # skip the fully-masked (tq < tk) columns of
                            # diagonal tiles
                            nc.tensor.matmul(
                                st[:, h * 512 + o:(h + 1) * 512],
                                kt_sb[hp][h * 64:(h + 1) * 64, i * 128:(i + 1) * 128],
                                qt_sb[hp][h * 64:(h + 1) * 64, J * 512 + o:(J + 1) * 512],
                                start=True, stop=True, tile_position=(h * 64, 0),
                            )
                        att = attp.tile([128, 1024], BF16, tag="att", name="att")
                        if o == 0:
                            nc.scalar.activation(att[:], st[:], EXP, scale=SCALE)
                        else:
                            # one strided exp covering both heads' valid spans
                            a3 = att[:].rearrange("p (h e) -> p h e", e=512)[:, :, o:512]
                            s3 = st[:].rearrange("p (h e) -> p h e", e=512)[:, :, o:512]
                            nc.scalar.activation(a3, s3, EXP, scale=SCALE)
                        if k >= 0:  # diagonal 128-block: keep tk <= tq
                            for h in range(2):
                                lo = h * 512 + k * 128
                                nc.gpsimd.tensor_mul(
                                    att[:, lo:lo + 128], att[:, lo:lo + 128], mask_sb[:]
                                )
                        atts.append(att)

                    def emit_av_mms(hp, J, jj, h, av, atts):
                        jq = 4 * J + jj
                        for i in range(jq + 1):
                            lhsT = atts[i][:, h * 512 + jj * 128:h * 512 + (jj + 1) * 128]
                            hl = hp * 2 + h
                            nc.tensor.matmul(
                                av[:, h * 65:(h + 1) * 65],
                                lhsT,
                                vaug_sb[i][:, hl * 65:(hl + 1) * 65],
                                start=(i == 0), stop=(i == jq),
                            )

                    def emit_av_finish(hp, J, jj, av):
                        onorm = onp.tile([128, 128], BF16, tag="on", name="on")
                        rc = rcp.tile([128, 2], F32, tag="rc", name="rc")
                        # both heads' denominators in one strided reciprocal
                        nc.vector.reciprocal(
                            rc[:], av[:].rearrange("p (h e) -> p h e", e=65)[:, :, 64:65])
                        for h in range(2):
                            nc.vector.tensor_scalar_mul(
                                onorm[:, h * 64:(h + 1) * 64],
                                av[:, h * 65:h * 65 + 64],
                                rc[:, h:h + 1],
                            )
                        tp = tppsp.tile([128, 128], BF16, tag="tp", name="tp")
                        nc.tensor.transpose(tp[:], onorm[:], ident[:])
                        nc.vector.tensor_copy(
                            outT_sb[hp][:, J * 512 + jj * 128:J * 512 + (jj + 1) * 128],
                            tp[:],
                        )

                    def make_av_items(hp, J, atts):
                        items = []
                        for jj in range(4):
                            av = avpsp.tile([128, 130], F32, tag="av", name="av")
                            for h in range(2):
                                items.append(
                                    lambda hp=hp, J=J, jj=jj, h=h, av=av, atts=atts:
                                    emit_av_mms(hp, J, jj, h, av, atts)
                                )
                            items.append(
                                lambda hp=hp, J=J, jj=jj, av=av:
                                emit_av_finish(hp, J, jj, av)
                            )
                        return items

                    work = [(hp, J) for hp in range(NCT) for J in range(NJ)]
                    av_queue = []
                    fpos = 0
                    for idx, (hp, J) in enumerate(work):
                        n_tk = 4 * J + 4
                        atts = []
                        # interleave: previous iteration's AV work + QKV
                        # fillers due up to two iterations out
                        due = []
                        while fpos < len(fillers) and fillers[fpos][0] <= idx + 2:
                            due.append(fillers[fpos][1])
                            fpos += 1
                        mixed = []
                        na, nd = len(av_queue), len(due)
                        ai = di = 0
                        for s in range(na + nd):
                            if ai * nd <= di * na and ai < na:
                                mixed.append(av_queue[ai]); ai += 1
                            elif di < nd:
                                mixed.append(due[di]); di += 1
                            else:
                                mixed.append(av_queue[ai]); ai += 1
                        total = len(mixed)
                        done = 0
                        for i in range(n_tk):
                            emit_qk_tile(hp, J, i, atts)
                            want = ((i + 1) * total) // n_tk
                            while done < want:
                                mixed[done]()
                                done += 1
                        while done < total:
                            mixed[done]()
                            done += 1
                        av_queue = make_av_items(hp, J, atts)
                    for c in av_queue:
                        c()

            # ---------------- Output projection (partial, own channels) ----
            with (
                tc.tile_pool(name="ysb", bufs=3) as ysbp,
                tc.tile_pool(name="yps", bufs=4, space="PSUM") as ypsp,
            ):
                for tt in range(NT):
                    yps = ypsp.tile([128, 1024], F32, tag="yps", name="yps")
                    for half in range(2):
                        for ct in range(NCT):
                            nc.tensor.matmul(
                                yps[:, half * 512:(half + 1) * 512],
                                outT_sb[ct][:, tt * 128:(tt + 1) * 128],
                                wo_sb[ct][:, half * 512:(half + 1) * 512],
                                start=(ct == 0), stop=(ct == NCT - 1),
                            )
                    ysb = ysbp.tile([128, 1024], BF16, tag="ysb", name="ysb")
                    nc.vector.tensor_copy(ysb[:], yps[:])
                    # split the write across 4 DMA engines (one engine moves
                    # only ~25GB/s; a whole 256KB tile would trail ~10us)
                    for q in range(4):
                        eng = nc.sync if q % 2 == 0 else nc.gpsimd
                        eng.dma_start(
                            y[tt * 128 + q * 32:tt * 128 + (q + 1) * 32, :],
                            ysb[q * 32:(q + 1) * 32, :])

    nc.compile()
    return nc


_NC_CACHE = {}


def _get_nc(T):
    if T not in _NC_CACHE:
        _NC_CACHE[T] = build_nc(T)
    return _NC_CACHE[T]


def shard_inputs(x, W_q, W_k, W_v, W_o):
    """Host-side sharding: per-core input dicts (bf16, transposed)."""
    T = x.shape[1]
    tri = np.triu(np.ones((128, 128), np.float32)).astype(NPBF16)
    in_maps = []
    for c in range(N_CORES):
        b, hg = c // 2, c % 2
        cs = slice(hg * CL, (hg + 1) * CL)
        in_maps.append({
            "xC": np.ascontiguousarray(
                x[b].T.reshape(D, T // 512, 512).transpose(1, 0, 2)
                .reshape(-1, 512)).astype(NPBF16),
            "wqT": np.ascontiguousarray(W_q[cs, :].T).astype(NPBF16),
            "wkT": np.ascontiguousarray(W_k[cs, :].T).astype(NPBF16),
            "wvT": np.ascontiguousarray(W_v[cs, :].T).astype(NPBF16),
            "woR": np.ascontiguousarray(W_o[:, cs].T).astype(NPBF16),
            "mask": tri,
        })
    return in_maps


def assemble_output(results, T):
    y = np.zeros((B, T, D), np.float32)
    for c in range(0, N_CORES, 2):
        b = c // 2
        y[b] = (results[c]["y"].astype(np.float32)
                + results[c + 1]["y"].astype(np.float32))
    return y


def kernel(x, W_q, W_k, W_v, W_o, _trace=False):
    x = np.asarray(x, dtype=np.float32)
    W_q = np.asarray(W_q, dtype=np.float32)
    W_k = np.asarray(W_k, dtype=np.float32)
    W_v = np.asarray(W_v, dtype=np.float32)
    W_o = np.asarray(W_o, dtype=np.float32)
    T = x.shape[1]
    nc = _get_nc(T)
    in_maps = shard_inputs(x, W_q, W_k, W_v, W_o)
    res = run_bass_kernel_spmd(
        nc, in_maps, core_ids=list(range(N_CORES)), trace=_trace
    )
    out = assemble_output(res.results, T)
    if _trace:
        return out, res
    return out
